# revision 22
# baseline (speedup 1.0000x reference)
"""BatchedFoveator Trainium2 kernel.

The reference computes an integral image (double cumsum) and gathers 4
corners per output pixel.  Mathematically that is exactly multi-scale
average pooling of the input image:

  level 0 (stride 1, 64 tokens): center crop  rows/cols [192, 320)
  level 1 (stride 2, 48 tokens): 2x2 average pool of  [128, 384)^2 (ring)
  level 2 (stride 4, 48 tokens): 4x4 average pool of full image   (ring)

Each level yields a 128x128 map per (b, c); token (gy, gx) of a level is
the 16x16 tile map[16gy:16gy+16, 16gx:16gx+16].  Ring order for levels
1/2: rows gy=0,1 (all gx), then gy=2..5 with gx in {0,1,6,7}, then
gy=6,7 (all gx).

Sharding: pure data parallel, batch 32 -> 4 images per core x 8 cores.

Per-core structure (4 images; loads and the DVE pooling chain pipeline
at channel granularity — DVE op time scales with the free-dim size, not
the partition count, so work is split along channels, never partitions):
  1. All input-only DMAs are issued first (image planes -> It, and the
     level-0 center crop DRAM -> U directly) so nothing queues behind a
     dependent transfer in a DGE ring FIFO.
  2. Per (b, c): DVE v-pair add -> P1v, h-pair add -> P1u (2x2 box
     sums, two pooled rows per partition).
  3. Level 1: ACT scales the P1u center crop -> M1pre (r2-major, rows
     pair-packed).  Full token rows gather straight from M1pre; middle
     rows go through two small rearrange DMAs -> M1 rows 32..96 (one
     map row per partition) -> column-compaction -> M1m.
  4. Level 2: DVE pool P1u again -> M4t; ACT scale -> M4 and compacted
     middles -> M4m.
  5. Gather DMAs (SBUF->SBUF) assemble U [p = b*32 + nh*16 + i]
     [c*1280 + n'*16 + j]  (nh = n//80, n' = n%80).
  6. Out DMAs per (b, nh, c) on the SWDGE path (HWDGE descriptor-gen
     for these 1280-descriptor DMAs stalls the ring ~7us each; the Q7
     CounterMachine emits descriptors 16 lanes in parallel).

DMA instruction issue cost (~0.6us HWDGE, ~1us SWDGE per dma_start) is
a main driver, so transfers are batched up to the 3-dim AP limit and
spread across the sync/scalar HWDGE rings and the gpsimd SWDGE path.
"""

import os
import threading

import numpy as np

N_CORES = 8
B_FULL = 32
B_SHARD = B_FULL // N_CORES  # 4
C = 3
S = 512
T = 16

_lock = threading.Lock()
_cache = {}


def _build_module():
    import concourse.bacc as bacc
    import concourse.mybir as mybir
    import concourse.tile as tile

    nc = bacc.Bacc("TRN2", target_bir_lowering=False, debug=False)
    f32 = mybir.dt.float32

    images = nc.dram_tensor("images", (B_SHARD, C, S, S), f32, kind="ExternalInput")
    out = nc.dram_tensor("out", (B_SHARD, 160, C, T, T), f32, kind="ExternalOutput")

    img = images.ap()
    outp = out.ap()

    with tile.TileContext(nc) as tc:
        with (
            tc.tile_pool(name="img", bufs=4) as pool_img,
            tc.tile_pool(name="p1v", bufs=2) as pool_p1v,
            tc.tile_pool(name="p1u", bufs=2) as pool_p1u,
            tc.tile_pool(name="m1pre", bufs=2) as pool_m1pre,
            tc.tile_pool(name="m1", bufs=2) as pool_m1,
            tc.tile_pool(name="m1m", bufs=2) as pool_m1m,
            tc.tile_pool(name="v4", bufs=2) as pool_v4,
            tc.tile_pool(name="m4t", bufs=2) as pool_m4t,
            tc.tile_pool(name="m4", bufs=2) as pool_m4,
            tc.tile_pool(name="m4m", bufs=2) as pool_m4m,
            tc.tile_pool(name="uout", bufs=1) as pool_u,
        ):
            # U holds the fully assembled output for all 4 images:
            # partition p = b*32 + nh*16 + i, free = c*1280 + n'*16 + j.
            U = pool_u.tile([128, C * 80 * T], f32, name="U")

            # ---- 1. input-only DMAs first ----
            its = []
            for b in range(B_SHARD):
                # It free: c*2048 + r*512 + x  (partition p = rows 4p..4p+4,
                # r = row%4 = 2*r2 + e); 8KB DRAM runs; per-channel DMAs so
                # the DVE chain can chase the load channel by channel.
                It = pool_img.tile([128, C * 2048], f32, name="It", tag="It")
                img_b = img[b].rearrange("c (p r) x -> p c (r x)", p=128)
                nc.sync.dma_start(out=It[0:64], in_=img_b[0:64])
                nc.scalar.dma_start(out=It[64:128], in_=img_b[64:128])
                its.append(It)

                # level 0: DRAM -> U directly, one DMA per channel, on the
                # SWDGE path which is otherwise idle until the out DMAs.
                # src rows 192+16gy+i, cols 192..320; dst tokens n = 8gy+gx.
                pbase = b * 32
                for c in range(C):
                    src = img[b, c, 192:320, 192:320].rearrange(
                        "(gy i) x -> i gy x", gy=8
                    )
                    dst = U[pbase : pbase + 16].rearrange(
                        "i (c f) -> i c f", c=C
                    )[:, c, 0 : 64 * 16].rearrange("i (gy f) -> i gy f", gy=8)
                    nc.gpsimd.dma_start(out=dst, in_=src)

            # ---- per-b pipeline ----
            for b in range(B_SHARD):
                It = its[b]

                def u_dst(n, wtok):
                    """U slice for tokens [n, n+wtok), all channels:
                    [i:16 p][c:3][w*16]."""
                    nh, np_ = divmod(n, 80)
                    pb = b * 32 + nh * 16
                    return U[pb : pb + 16].rearrange(
                        "i (c f) -> i c f", c=C
                    )[:, :, np_ * 16 : np_ * 16 + wtok * 16]

                P1v = pool_p1v.tile([128, C * 1024], f32, name="P1v", tag="P1v")
                P1u = pool_p1u.tile([128, C * 512], f32, name="P1u", tag="P1u")
                M1pre = pool_m1pre.tile(
                    [128, 2 * C * 128], f32, name="M1pre", tag="M1pre"
                )
                M1 = pool_m1.tile([128, C * 128], f32, name="M1", tag="M1")
                M1m = pool_m1m.tile([128, C * 64], f32, name="M1m", tag="M1m")
                V4 = pool_v4.tile([128, C * 256], f32, name="V4", tag="V4")
                M4t = pool_m4t.tile([128, C * 128], f32, name="M4t", tag="M4t")
                M4 = pool_m4.tile([128, C * 128], f32, name="M4", tag="M4")
                M4m = pool_m4m.tile([128, C * 64], f32, name="M4m", tag="M4m")

                it_v = It.rearrange("p (cr e x) -> p cr e x", cr=2 * C, e=2)
                p1v_v3 = P1v.rearrange("p (cr x) -> p cr x", cr=2 * C)
                p1v_h = P1v.rearrange("p (cr xp par) -> p cr xp par", cr=2 * C, par=2)
                p1u_v = P1u.rearrange("p (cr xp) -> p cr xp", cr=2 * C)
                p1u_c4 = P1u.rearrange("p (c r2 xp) -> p c r2 xp", c=C, r2=2)
                m1pre_v = M1pre.rearrange("p (r2 cx) -> p r2 cx", r2=2)
                m1pre_c = M1pre.rearrange("p (r2 c x) -> p r2 c x", r2=2, c=C)
                v4_v = V4.rearrange("p (c xp) -> p c xp", c=C)
                v4_h = V4.rearrange("p (c X par) -> p c X par", c=C, par=2)
                m4t_v = M4t.rearrange("p (c X) -> p c X", c=C)
                m1_c = M1.rearrange("p (c X) -> p c X", c=C)
                m4_v = M4.rearrange("p (c X) -> p c X", c=C)
                m4m_v = M4m.rearrange("p (c s j) -> p c s j", c=C, s=2)
                m1m_v = M1m.rearrange("p (c s j) -> p c s j", c=C, s=2)

                # ---- 2.-4. pooling chain ----
                nc.vector.tensor_add(
                    out=p1v_v3, in0=it_v[:, :, 0], in1=it_v[:, :, 1]
                )
                nc.vector.tensor_add(
                    out=p1u_v, in0=p1v_h[:, :, :, 0], in1=p1v_h[:, :, :, 1]
                )
                # level-1 scaled crop (rows = P1 rows 64..192 live on
                # partitions 32..96; engine APs from base 32/64 may span
                # at most 32 partitions)
                for r2 in range(2):
                    for lo in (32, 64):
                        nc.scalar.mul(
                            m1pre_v[lo : lo + 32, r2].rearrange(
                                "p (c x) -> p c x", c=C
                            ),
                            p1u_c4[lo : lo + 32, :, r2, 64:192],
                            0.25,
                        )
                # level-2 second pooling stage
                nc.vector.tensor_add(
                    out=v4_v, in0=p1u_c4[:, :, 0], in1=p1u_c4[:, :, 1]
                )
                nc.vector.tensor_add(
                    out=m4t_v, in0=v4_h[:, :, :, 0], in1=v4_h[:, :, :, 1]
                )
                nc.scalar.mul(M4[:], M4t[:], 1.0 / 16.0)

                # middle map rows 32..96 of M1, one row per partition
                # (partition-pair split must live on the DMA src side)
                for h in range(2):
                    nc.scalar.dma_start(
                        out=M1[32 + 32 * h : 64 + 32 * h],
                        in_=M1pre[48 + 16 * h : 64 + 16 * h].rearrange(
                            "p (r2 cx) -> p r2 cx", r2=2
                        ),
                    )
                # compacted ring-middle columns
                for side, x0 in ((0, 0), (1, 96)):
                    for lo in (32, 64):
                        nc.scalar.mul(
                            m4m_v[lo : lo + 32, :, side],
                            m4t_v[lo : lo + 32, :, x0 : x0 + 32],
                            1.0 / 16.0,
                        )
                        nc.vector.tensor_scalar_mul(
                            m1m_v[lo : lo + 32, :, side],
                            m1_c[lo : lo + 32, :, x0 : x0 + 32],
                            1.0,
                        )

                # ---- 5.+6. gathers, with out DMAs interleaved as soon as
                # each nh half of U is complete ----
                def emit_out(nh):
                    pb = b * 32 + nh * 16
                    u_src = U[pb : pb + 16].rearrange(
                        "i (c n j) -> i c n j", c=C, n=80
                    )
                    for c in range(C):
                        nc.gpsimd.dma_start(
                            out=outp[b, 80 * nh : 80 * nh + 80, c].transpose(
                                [1, 0, 2]
                            ),
                            in_=u_src[:, c],
                        )

                # level-1 full rows gy 0,1 (tokens 64..80) complete nh0
                # (tokens 0..64 are the level-0 crop, already DMA'd)
                for gy in (0, 1):
                    src = M1pre[32 + 8 * gy : 40 + 8 * gy].rearrange(
                        "p (r2 cx) -> p r2 cx", r2=2
                    )
                    nc.sync.dma_start(out=u_dst(64 + 8 * gy, 8), in_=src)
                emit_out(0)
                # rest of level 1: full rows gy 6,7 and middles
                for gy in (6, 7):
                    src = M1pre[32 + 8 * gy : 40 + 8 * gy].rearrange(
                        "p (r2 cx) -> p r2 cx", r2=2
                    )
                    nc.sync.dma_start(out=u_dst(64 + 32 + 8 * (gy - 6), 8), in_=src)
                for gy in (2, 3, 4, 5):
                    src1 = M1m[16 * gy : 16 * gy + 16].rearrange(
                        "i (c x) -> i c x", c=C
                    )
                    nc.sync.dma_start(out=u_dst(64 + 16 + 4 * (gy - 2), 4), in_=src1)
                # level 2: full rows and middles
                for gy in (0, 1, 6, 7):
                    k = (8 * gy) if gy < 2 else (32 + 8 * (gy - 6))
                    src = M4[16 * gy : 16 * gy + 16].rearrange(
                        "i (c x) -> i c x", c=C
                    )
                    nc.scalar.dma_start(out=u_dst(112 + k, 8), in_=src)
                for gy in (2, 3, 4, 5):
                    src4 = M4m[16 * gy : 16 * gy + 16].rearrange(
                        "i (c x) -> i c x", c=C
                    )
                    nc.scalar.dma_start(out=u_dst(112 + 16 + 4 * (gy - 2), 4), in_=src4)
                emit_out(1)

    nc.compile()
    return nc


def _get_module():
    with _lock:
        if "nc" not in _cache:
            _cache["nc"] = _build_module()
        return _cache["nc"]


def kernel(images: np.ndarray) -> np.ndarray:
    from concourse.bass_utils import run_bass_kernel_spmd

    images = np.ascontiguousarray(np.asarray(images, dtype=np.float32))
    assert images.shape == (B_FULL, C, S, S), images.shape

    nc = _get_module()
    in_maps = [
        {"images": images[k * B_SHARD : (k + 1) * B_SHARD]} for k in range(N_CORES)
    ]
    res = run_bass_kernel_spmd(
        nc,
        in_maps,
        core_ids=list(range(N_CORES)),
        trace=bool(int(os.environ.get("FOV_TRACE", "0"))),
    )
    _cache["last_results"] = res
    out = np.concatenate([r["out"] for r in res.results], axis=0)
    return out


if __name__ == "__main__":
    x = np.random.randn(B_FULL, C, S, S).astype(np.float32)
    y = kernel(x)
    print("out", y.shape, y.dtype, float(np.abs(y).max()))


# revision 23
# speedup vs baseline: 1.2577x; 1.2577x over previous
"""BatchedFoveator Trainium2 kernel.

The reference computes an integral image (double cumsum) and gathers 4
corners per output pixel.  Mathematically that is exactly multi-scale
average pooling of the input image:

  level 0 (stride 1, 64 tokens): center crop  rows/cols [192, 320)
  level 1 (stride 2, 48 tokens): 2x2 average pool of  [128, 384)^2 (ring)
  level 2 (stride 4, 48 tokens): 4x4 average pool of full image   (ring)

Each level yields a 128x128 map per (b, c); token (gy, gx) of a level is
the 16x16 tile map[16gy:16gy+16, 16gx:16gx+16].  Ring order for levels
1/2: rows gy=0,1 (all gx), then gy=2..5 with gx in {0,1,6,7}, then
gy=6,7 (all gx).

Sharding: pure data parallel, batch 32 -> 4 images per core x 8 cores.

Per-core structure (4 images; loads and the DVE pooling chain pipeline
at channel granularity — DVE op time scales with the free-dim size, not
the partition count, so work is split along channels, never partitions):
  1. All input-only DMAs are issued first (image planes -> It, and the
     level-0 center crop DRAM -> U directly) so nothing queues behind a
     dependent transfer in a DGE ring FIFO.
  2. Per (b, c): DVE v-pair add -> P1v, h-pair add -> P1u (2x2 box
     sums, two pooled rows per partition).
  3. Level 1: ACT scales the P1u center crop -> M1pre (r2-major, rows
     pair-packed).  Full token rows gather straight from M1pre; middle
     rows go through two small rearrange DMAs -> M1 rows 32..96 (one
     map row per partition) -> column-compaction -> M1m.
  4. Level 2: DVE pool P1u again -> M4t; ACT scale -> M4 and compacted
     middles -> M4m.
  5. Gather DMAs (SBUF->SBUF) assemble U [p = b*32 + nh*16 + i]
     [c*1280 + n'*16 + j]  (nh = n//80, n' = n%80).
  6. Out DMAs per (b, nh, c) on the SWDGE path (HWDGE descriptor-gen
     for these 1280-descriptor DMAs stalls the ring ~7us each; the Q7
     CounterMachine emits descriptors 16 lanes in parallel).

DMA instruction issue cost (~0.6us HWDGE, ~1us SWDGE per dma_start) is
a main driver, so transfers are batched up to the 3-dim AP limit and
spread across the sync/scalar HWDGE rings and the gpsimd SWDGE path.
"""

import os
import threading

import numpy as np

N_CORES = 8
B_FULL = 32
B_SHARD = B_FULL // N_CORES  # 4
C = 3
S = 512
T = 16

_lock = threading.Lock()
_cache = {}


def _build_module():
    import concourse.bacc as bacc
    import concourse.mybir as mybir
    import concourse.tile as tile

    nc = bacc.Bacc("TRN2", target_bir_lowering=False, debug=False)
    f32 = mybir.dt.float32

    images = nc.dram_tensor("images", (B_SHARD, C, S, S), f32, kind="ExternalInput")
    out = nc.dram_tensor("out", (B_SHARD, 160, C, T, T), f32, kind="ExternalOutput")

    img = images.ap()
    outp = out.ap()

    with tile.TileContext(nc) as tc:
        with (
            tc.tile_pool(name="img", bufs=4) as pool_img,
            tc.tile_pool(name="p1v", bufs=2) as pool_p1v,
            tc.tile_pool(name="p1u", bufs=2) as pool_p1u,
            tc.tile_pool(name="m1pre", bufs=2) as pool_m1pre,
            tc.tile_pool(name="m1", bufs=2) as pool_m1,
            tc.tile_pool(name="m1m", bufs=2) as pool_m1m,
            tc.tile_pool(name="v4", bufs=2) as pool_v4,
            tc.tile_pool(name="m4t", bufs=2) as pool_m4t,
            tc.tile_pool(name="m4", bufs=2) as pool_m4,
            tc.tile_pool(name="m4m", bufs=2) as pool_m4m,
            tc.tile_pool(name="uout", bufs=1) as pool_u,
        ):
            # U holds the fully assembled output for all 4 images:
            # partition p = b*32 + nh*16 + i, free = c*1280 + n'*16 + j.
            U = pool_u.tile([128, C * 80 * T], f32, name="U")

            # ---- 1. input-only DMAs first ----
            its = []
            for b in range(B_SHARD):
                # It free: c*2048 + r*512 + x  (partition p = rows 4p..4p+4,
                # r = row%4 = 2*r2 + e); 8KB DRAM runs; per-channel DMAs so
                # the DVE chain can chase the load channel by channel.
                It = pool_img.tile([128, C * 2048], f32, name="It", tag="It")
                img_b = img[b].rearrange("c (p r) x -> p c (r x)", p=128)
                nc.sync.dma_start(out=It[:], in_=img_b)
                its.append(It)

                # level 0: DRAM -> U directly, one DMA per channel, on the
                # SWDGE path which is otherwise idle until the out DMAs.
                # src rows 192+16gy+i, cols 192..320; dst tokens n = 8gy+gx.
                pbase = b * 32
                for c in range(C):
                    src = img[b, c, 192:320, 192:320].rearrange(
                        "(gy i) x -> i gy x", gy=8
                    )
                    dst = U[pbase : pbase + 16].rearrange(
                        "i (c f) -> i c f", c=C
                    )[:, c, 0 : 64 * 16].rearrange("i (gy f) -> i gy f", gy=8)
                    nc.gpsimd.dma_start(out=dst, in_=src)

            # ---- per-b pipeline ----
            for b in range(B_SHARD):
                It = its[b]

                def u_dst(n, wtok):
                    """U slice for tokens [n, n+wtok), all channels:
                    [i:16 p][c:3][w*16]."""
                    nh, np_ = divmod(n, 80)
                    pb = b * 32 + nh * 16
                    return U[pb : pb + 16].rearrange(
                        "i (c f) -> i c f", c=C
                    )[:, :, np_ * 16 : np_ * 16 + wtok * 16]

                P1v = pool_p1v.tile([128, C * 1024], f32, name="P1v", tag="P1v")
                P1u = pool_p1u.tile([128, C * 512], f32, name="P1u", tag="P1u")
                M1pre = pool_m1pre.tile(
                    [128, 2 * C * 128], f32, name="M1pre", tag="M1pre"
                )
                M1 = pool_m1.tile([128, C * 128], f32, name="M1", tag="M1")
                M1m = pool_m1m.tile([128, C * 64], f32, name="M1m", tag="M1m")
                V4 = pool_v4.tile([128, C * 256], f32, name="V4", tag="V4")
                M4t = pool_m4t.tile([128, C * 128], f32, name="M4t", tag="M4t")
                M4 = pool_m4.tile([128, C * 128], f32, name="M4", tag="M4")
                M4m = pool_m4m.tile([128, C * 64], f32, name="M4m", tag="M4m")

                it_v = It.rearrange("p (cr e x) -> p cr e x", cr=2 * C, e=2)
                p1v_v3 = P1v.rearrange("p (cr x) -> p cr x", cr=2 * C)
                p1v_h = P1v.rearrange("p (cr xp par) -> p cr xp par", cr=2 * C, par=2)
                p1u_v = P1u.rearrange("p (cr xp) -> p cr xp", cr=2 * C)
                p1u_c4 = P1u.rearrange("p (c r2 xp) -> p c r2 xp", c=C, r2=2)
                m1pre_v = M1pre.rearrange("p (r2 cx) -> p r2 cx", r2=2)
                m1pre_c = M1pre.rearrange("p (r2 c x) -> p r2 c x", r2=2, c=C)
                v4_v = V4.rearrange("p (c xp) -> p c xp", c=C)
                v4_h = V4.rearrange("p (c X par) -> p c X par", c=C, par=2)
                m4t_v = M4t.rearrange("p (c X) -> p c X", c=C)
                m1_c = M1.rearrange("p (c X) -> p c X", c=C)
                m4_v = M4.rearrange("p (c X) -> p c X", c=C)
                m4m_v = M4m.rearrange("p (c s j) -> p c s j", c=C, s=2)
                m1m_v = M1m.rearrange("p (c s j) -> p c s j", c=C, s=2)

                # ---- 2.-4. pooling chain ----
                nc.vector.tensor_add(
                    out=p1v_v3, in0=it_v[:, :, 0], in1=it_v[:, :, 1]
                )
                nc.vector.tensor_add(
                    out=p1u_v, in0=p1v_h[:, :, :, 0], in1=p1v_h[:, :, :, 1]
                )
                # level-1 scaled crop (rows = P1 rows 64..192 live on
                # partitions 32..96; engine APs from base 32/64 may span
                # at most 32 partitions)
                for r2 in range(2):
                    for lo in (32, 64):
                        nc.scalar.mul(
                            m1pre_v[lo : lo + 32, r2].rearrange(
                                "p (c x) -> p c x", c=C
                            ),
                            p1u_c4[lo : lo + 32, :, r2, 64:192],
                            0.25,
                        )
                # level-2 second pooling stage
                nc.vector.tensor_add(
                    out=v4_v, in0=p1u_c4[:, :, 0], in1=p1u_c4[:, :, 1]
                )
                nc.vector.tensor_add(
                    out=m4t_v, in0=v4_h[:, :, :, 0], in1=v4_h[:, :, :, 1]
                )
                nc.scalar.mul(M4[:], M4t[:], 1.0 / 16.0)

                # middle map rows 32..96 of M1, one row per partition
                # (partition-pair split must live on the DMA src side)
                for h in range(2):
                    nc.scalar.dma_start(
                        out=M1[32 + 32 * h : 64 + 32 * h],
                        in_=M1pre[48 + 16 * h : 64 + 16 * h].rearrange(
                            "p (r2 cx) -> p r2 cx", r2=2
                        ),
                    )
                # compacted ring-middle columns
                for side, x0 in ((0, 0), (1, 96)):
                    for lo in (32, 64):
                        nc.scalar.mul(
                            m4m_v[lo : lo + 32, :, side],
                            m4t_v[lo : lo + 32, :, x0 : x0 + 32],
                            1.0 / 16.0,
                        )
                        nc.vector.tensor_scalar_mul(
                            m1m_v[lo : lo + 32, :, side],
                            m1_c[lo : lo + 32, :, x0 : x0 + 32],
                            1.0,
                        )

                # ---- 5.+6. gathers, with out DMAs interleaved as soon as
                # each nh half of U is complete ----
                def emit_out(nh):
                    pb = b * 32 + nh * 16
                    u_src = U[pb : pb + 16].rearrange(
                        "i (c n j) -> i c n j", c=C, n=80
                    )
                    for c in range(C):
                        nc.gpsimd.dma_start(
                            out=outp[b, 80 * nh : 80 * nh + 80, c].transpose(
                                [1, 0, 2]
                            ),
                            in_=u_src[:, c],
                        )

                # level-1 full rows gy 0,1 (tokens 64..80) complete nh0
                # (tokens 0..64 are the level-0 crop, already DMA'd)
                for gy in (0, 1):
                    src = M1pre[32 + 8 * gy : 40 + 8 * gy].rearrange(
                        "p (r2 cx) -> p r2 cx", r2=2
                    )
                    nc.sync.dma_start(out=u_dst(64 + 8 * gy, 8), in_=src)
                emit_out(0)
                # rest of level 1: full rows gy 6,7 and middles
                for gy in (6, 7):
                    src = M1pre[32 + 8 * gy : 40 + 8 * gy].rearrange(
                        "p (r2 cx) -> p r2 cx", r2=2
                    )
                    nc.sync.dma_start(out=u_dst(64 + 32 + 8 * (gy - 6), 8), in_=src)
                for gy in (2, 3, 4, 5):
                    src1 = M1m[16 * gy : 16 * gy + 16].rearrange(
                        "i (c x) -> i c x", c=C
                    )
                    nc.sync.dma_start(out=u_dst(64 + 16 + 4 * (gy - 2), 4), in_=src1)
                # level 2: full rows and middles
                for gy in (0, 1, 6, 7):
                    k = (8 * gy) if gy < 2 else (32 + 8 * (gy - 6))
                    src = M4[16 * gy : 16 * gy + 16].rearrange(
                        "i (c x) -> i c x", c=C
                    )
                    nc.scalar.dma_start(out=u_dst(112 + k, 8), in_=src)
                for gy in (2, 3, 4, 5):
                    src4 = M4m[16 * gy : 16 * gy + 16].rearrange(
                        "i (c x) -> i c x", c=C
                    )
                    nc.scalar.dma_start(out=u_dst(112 + 16 + 4 * (gy - 2), 4), in_=src4)
                emit_out(1)

    nc.compile()
    return nc


def _get_module():
    with _lock:
        if "nc" not in _cache:
            _cache["nc"] = _build_module()
        return _cache["nc"]


def kernel(images: np.ndarray) -> np.ndarray:
    from concourse.bass_utils import run_bass_kernel_spmd

    images = np.ascontiguousarray(np.asarray(images, dtype=np.float32))
    assert images.shape == (B_FULL, C, S, S), images.shape

    nc = _get_module()
    in_maps = [
        {"images": images[k * B_SHARD : (k + 1) * B_SHARD]} for k in range(N_CORES)
    ]
    res = run_bass_kernel_spmd(
        nc,
        in_maps,
        core_ids=list(range(N_CORES)),
        trace=bool(int(os.environ.get("FOV_TRACE", "0"))),
    )
    _cache["last_results"] = res
    out = np.concatenate([r["out"] for r in res.results], axis=0)
    return out


if __name__ == "__main__":
    x = np.random.randn(B_FULL, C, S, S).astype(np.float32)
    y = kernel(x)
    print("out", y.shape, y.dtype, float(np.abs(y).max()))
